# revision 14
# baseline (speedup 1.0000x reference)
"""Trainium2 Bass kernel for nn_CapsuleLinear (k-means 'dot' routing, 3 iters).

Math (per example b):
  priors[o,i,v] = sum_l W[o,i,v,l] * x[b,i,l]
  out0 = mean_i priors
  3x: n = normalize(out); logits[o,i] = sum_v priors*n; probs = softmax_o(logits);
      out[o,v] = sum_i probs*priors
  result = squash(out) + bias

Sharding: data-parallel over batch B=64 across 8 cores (8 examples/core).

Per-core layout (P = 128 partitions = (i_p in 0..15, b in 0..7), p = i_p*8+b):
  priors SBUF fp16 [128, ib=32, v=16, o=64], full i = ib*16 + i_p.
  Phase 1: priors via PE matmuls (block-diag x as lhsT, W2[ib] as rhs);
  PSUM -> SBUF copies alternate ACT/DVE; out0 = sum_i priors accumulates
  entirely on the PE via delta_{b,b'} ones-matmuls (DVE stays empty so the
  phase is PE/DMA-bound).
  Iterations use raw logits: rawlog = priors . out (unnormalized), then one
  small scale by rn = 1/||out|| (computed as exp(-0.5 ln nsq) so the ACT
  engine never switches activation tables; softmax shift invariance is not
  affected since the scale is exact). elog/logits fp16. The out-phase
  interleaves DVE muls (8-ib chunks) with batched PE ones-matmuls so the PE
  queue stays full and ramps to its fast p-state.
"""

import os

import numpy as np

import concourse.bacc as bacc
import concourse.tile as tile
from concourse import mybir
from concourse.bass_utils import run_bass_kernel_spmd

B, I, O, V, L = 64, 512, 64, 16, 8
NCORES = 8
BL = B // NCORES  # 8 examples per core
IB = I // 16  # 32 blocks of 16 i's
IP = 16  # i_p values per partition group

f32 = mybir.dt.float32
f16 = mybir.dt.float16

LAST_RESULT = None  # stash of BassKernelResults for test harness


def _build_kernel():
    nc = bacc.Bacc(
        "TRN2",
        target_bir_lowering=False,
        debug=False,
        enable_asserts=False,
        num_devices=NCORES,
    )
    w2_d = nc.dram_tensor("w2", [IB, 128, O * V], f16, kind="ExternalInput")
    xdg_d = nc.dram_tensor("xdg", [128, IB, 128], f16, kind="ExternalInput")
    ones_d = nc.dram_tensor("onesd", [128, 128], f16, kind="ExternalInput")
    bias_d = nc.dram_tensor("biasT", [V, O], f32, kind="ExternalInput")
    out_d = nc.dram_tensor("out", [BL, V, O], f32, kind="ExternalOutput")

    with tile.TileContext(nc) as tc:
        _body(nc, tc, w2_d, xdg_d, ones_d, bias_d, out_d)
    nc.compile()
    return nc


def _body(nc, tc, w2_d, xdg_d, ones_d, bias_d, out_d):
    AL = mybir.AluOpType
    X = mybir.AxisListType.X
    AF = mybir.ActivationFunctionType

    from contextlib import ExitStack

    with ExitStack() as ctx:
        big = ctx.enter_context(tc.tile_pool(name="big", bufs=1))
        wp = ctx.enter_context(tc.tile_pool(name="wp", bufs=6))
        xp = ctx.enter_context(tc.tile_pool(name="xp", bufs=4))
        sm = ctx.enter_context(tc.tile_pool(name="sm", bufs=1))
        pr_ps = ctx.enter_context(tc.tile_pool(name="prps", bufs=3, space="PSUM"))
        out_ps = ctx.enter_context(tc.tile_pool(name="outps", bufs=1, space="PSUM"))

        # ---- persistent tiles ----
        priors = big.tile([128, IB, V, O], f16)
        prod = big.tile([128, IB, V, O], f16)
        logits = big.tile([128, IB, O], f16)
        elog = big.tile([128, IB, O], f16)
        probs = big.tile([128, IB, O], f16)
        ones_t = big.tile([128, 128], f16)
        bias_t = big.tile([BL, V, O], f32)
        out_sb = big.tile([128, V, O], f16)
        sqv = big.tile([128, V, O], f32)
        nsq = big.tile([128, O], f32)
        lnq = big.tile([128, O], f32)
        rn = big.tile([128, O], f16)
        zs = big.tile([128, IB], f32)
        rz = big.tile([128, IB], f32)

        # all 32 xdg tiles in ONE DMA (per-descriptor overhead is ~600ns, so
        # 32 small DMAs would serialize ~19us of queue time); issue it on the
        # ACT engine's DMA queue so the sync queue starts streaming w2
        # immediately.
        xdg_all = big.tile([128, IB, 128], f16)
        nc.scalar.dma_start(out=xdg_all[:], in_=xdg_d[:])
        nc.scalar.dma_start(out=ones_t[:], in_=ones_d[:])
        nc.scalar.dma_start(
            out=bias_t[:], in_=bias_d[:].unsqueeze(0).broadcast_to([BL, V, O])
        )

        # ---- phase 1: priors + out0 (out0 entirely on PE) ----
        out0 = out_ps.tile([128, V, O], f32)
        out0f = out0[:].rearrange("p v o -> p (v o)")

        def out0_mms(ib_lo, ib_hi):
            for ib in range(ib_lo, ib_hi):
                pslc = priors[:, ib].rearrange("p v o -> p (v o)")
                for h in range(2):
                    sl = slice(h * 512, (h + 1) * 512)
                    nc.tensor.matmul(
                        out0f[:, sl],
                        ones_t[:],
                        pslc[:, sl],
                        start=(ib == 0),
                        stop=(ib == IB - 1),
                        skip_group_check=True,
                    )

        for ib in range(IB):
            w = wp.tile([128, O * V], f16, tag="w")
            nc.sync.dma_start(out=w[:], in_=w2_d[ib])
            pp = pr_ps.tile([128, O * V], f32, tag="pp")
            for h in range(2):
                sl = slice(h * 512, (h + 1) * 512)
                nc.tensor.matmul(
                    pp[:, sl], xdg_all[:, ib], w[:, sl], start=True, stop=True
                )
            # PSUM (o,v) -> SBUF priors[:, ib] in (v, o) order, cast to fp16;
            # alternate ACT/DVE so neither engine is the phase bottleneck.
            ppv = pp[:].rearrange("p (o v) -> p v o", o=O)
            if ib % 2 == 0:
                nc.scalar.copy(out=priors[:, ib], in_=ppv)
            else:
                nc.vector.tensor_copy(out=priors[:, ib], in_=ppv)
            # out0 ones-matmuls lag 4 ibs behind so their copy deps are long
            # done when the PE reaches them (no pipeline stall).
            if ib % 4 == 3 and ib >= 7:
                out0_mms(ib - 7, ib - 3)
        out0_mms(IB - 4, IB)

        # seed routing state: out_sb = out0 (scale handled by rn)
        nc.scalar.copy(out=out_sb[:], in_=out0[:])

        # ---- phase 2: routing iterations ----
        # o-split between DVE and GpSimd: the Pool engine (otherwise idle,
        # ~0.5 G elem/s/lane) carries o[OS:64] of every big elementwise pass
        # as an independent lane until the L4 join.
        OS = 52
        for t in range(3):
            # rawlog mul chunk 0 first so the DVE never stalls on ACT square
            nc.gpsimd.tensor_mul(
                prod[:, :, :, OS:O],
                priors[:, :, :, OS:O],
                out_sb[:, :, OS:O].unsqueeze(1).broadcast_to([128, IB, V, O - OS]),
            )
            nc.vector.tensor_mul(
                prod[:, 0:16, :, 0:OS],
                priors[:, 0:16, :, 0:OS],
                out_sb[:, :, 0:OS].unsqueeze(1).broadcast_to([128, 16, V, OS]),
            )
            # rn = 1/||out|| via ln+exp (no activation-table switches)
            nc.scalar.square(sqv[:], out_sb[:])
            nsqv = sqv[:].transpose([0, 2, 1])  # [p, O, V] strided view
            nc.vector.tensor_reduce(out=nsq[:], in_=nsqv, axis=X, op=AL.add)
            nc.scalar.activation(out=lnq[:], in_=nsq[:], func=AF.Ln)
            nc.scalar.activation(out=rn[:], in_=lnq[:], func=AF.Exp, scale=-0.5)
            nc.vector.tensor_mul(
                prod[:, 16:32, :, 0:OS],
                priors[:, 16:32, :, 0:OS],
                out_sb[:, :, 0:OS].unsqueeze(1).broadcast_to([128, 16, V, OS]),
            )
            # v-reduction tree (fp16, 2x mode on DVE; Pool lane in step)
            nc.gpsimd.tensor_add(
                prod[:, :, 0:8, OS:O], prod[:, :, 0:8, OS:O], prod[:, :, 8:16, OS:O]
            )
            nc.vector.tensor_add(
                prod[:, :, 0:8, 0:OS], prod[:, :, 0:8, 0:OS], prod[:, :, 8:16, 0:OS]
            )
            nc.gpsimd.tensor_add(
                prod[:, :, 0:4, OS:O], prod[:, :, 0:4, OS:O], prod[:, :, 4:8, OS:O]
            )
            nc.vector.tensor_add(
                prod[:, :, 0:4, 0:OS], prod[:, :, 0:4, 0:OS], prod[:, :, 4:8, 0:OS]
            )
            nc.gpsimd.tensor_add(
                prod[:, :, 0:2, OS:O], prod[:, :, 0:2, OS:O], prod[:, :, 2:4, OS:O]
            )
            nc.vector.tensor_add(
                prod[:, :, 0:2, 0:OS], prod[:, :, 0:2, 0:OS], prod[:, :, 2:4, 0:OS]
            )
            # logits = (sum_v prod) * rn ; softmax over o, split in ib-halves
            # so ACT exp overlaps the DVE tail.
            for hh in range(2):
                si = slice(hh * 16, (hh + 1) * 16)
                nc.vector.tensor_add(logits[:, si], prod[:, si, 0], prod[:, si, 1])
                nc.vector.tensor_mul(
                    logits[:, si],
                    logits[:, si],
                    rn[:].unsqueeze(1).broadcast_to([128, 16, O]),
                )
                nc.scalar.activation(out=elog[:, si], in_=logits[:, si], func=AF.Exp)
                nc.vector.tensor_reduce(
                    out=zs[:, si], in_=elog[:, si], axis=X, op=AL.add
                )
            nc.vector.reciprocal(rz[:], zs[:])
            nc.vector.tensor_mul(
                probs[:], elog[:], rz[:].unsqueeze(2).broadcast_to([128, IB, O])
            )

            # out_new[p, v, o] = sum_i probs * priors: DVE muls in 8-ib
            # chunks, PE ones-matmuls batched per chunk (PE queue stays full
            # -> fast p-state; it rides ~2us behind the DVE).
            out_new = pr_ps.tile([128, V, O], f32, tag="pp")
            onf = out_new[:].rearrange("p v o -> p (v o)")
            # chunk sizes taper so the PE's last matmul batch lands right
            # after the DVE's last mul (small tail)
            bounds = [0, 10, 20, 26, 32]
            for c in range(4):
                lo, hi = bounds[c], bounds[c + 1]
                s = slice(lo, hi)
                nc.gpsimd.tensor_mul(
                    prod[:, s, :, OS:O],
                    priors[:, s, :, OS:O],
                    probs[:, s, OS:O].unsqueeze(2).broadcast_to(
                        [128, hi - lo, V, O - OS]
                    ),
                )
                nc.vector.tensor_mul(
                    prod[:, s, :, 0:OS],
                    priors[:, s, :, 0:OS],
                    probs[:, s, 0:OS].unsqueeze(2).broadcast_to(
                        [128, hi - lo, V, OS]
                    ),
                )
                for ib in range(lo, hi):
                    pslc = prod[:, ib].rearrange("p v o -> p (v o)")
                    for h in range(2):
                        sl = slice(h * 512, (h + 1) * 512)
                        nc.tensor.matmul(
                            onf[:, sl],
                            ones_t[:],
                            pslc[:, sl],
                            start=(ib == 0),
                            stop=(ib == IB - 1),
                            skip_group_check=True,
                        )
            if t < 2:
                nc.scalar.copy(out=out_sb[:], in_=out_new[:])
            else:
                out_prev = out_new

        # ---- squash + bias on partitions 0..7 (b rows) ----
        out3 = sm.tile([BL, V, O], f32, tag="out3")
        nc.scalar.copy(out=out3[:], in_=out_prev[0:BL])
        nc.scalar.square(sqv[0:BL], out3[:])
        nsqv3 = sqv[0:BL].transpose([0, 2, 1])
        nc.vector.tensor_reduce(out=nsq[0:BL], in_=nsqv3, axis=X, op=AL.add)
        nc.scalar.activation(out=lnq[0:BL], in_=nsq[0:BL], func=AF.Ln)
        norm3 = sm.tile([BL, O], f32, tag="norm3")
        nc.scalar.activation(out=norm3[:], in_=lnq[0:BL], func=AF.Exp, scale=0.5)
        den = sm.tile([BL, O], f32, tag="den")
        nc.vector.tensor_scalar_add(den[:], nsq[0:BL], 1.0)
        rden = sm.tile([BL, O], f32, tag="rden")
        nc.vector.reciprocal(rden[:], den[:])
        scl = sm.tile([BL, O], f32, tag="scl")
        nc.vector.tensor_mul(scl[:], norm3[:], rden[:])

        outf = sm.tile([BL, V, O], f32, tag="outf")
        nc.vector.tensor_mul(
            outf[:], out3[:], scl[:].unsqueeze(1).broadcast_to([BL, V, O])
        )
        nc.vector.tensor_add(outf[:], outf[:], bias_t[:])
        nc.sync.dma_start(out=out_d[:], in_=outf[:])


_NC_CACHE = []


def _get_nc():
    if not _NC_CACHE:
        _NC_CACHE.append(_build_kernel())
    return _NC_CACHE[0]


def kernel(x, weight, bias):
    global LAST_RESULT
    x = np.asarray(x, dtype=np.float32)
    weight = np.asarray(weight, dtype=np.float32)
    bias = np.asarray(bias, dtype=np.float32)

    # W2[ib, (i_sub, l), (o, v)] = W[o, ib*16+i_sub, v, l]  (fp16: same byte
    # cost as bf16 but 4x finer mantissa; values are well within fp16 range)
    w2 = (
        np.ascontiguousarray(weight.transpose(1, 3, 0, 2))
        .reshape(IB, 128, O * V)
        .astype(np.float16)
    )
    biasT = np.ascontiguousarray(bias.T)  # [V, O]

    idx = np.arange(128)
    onesd = (idx[:, None] % BL == idx[None, :] % BL).astype(np.float16)

    in_maps = []
    for c in range(NCORES):
        xc = x[c * BL : (c + 1) * BL]  # [BL, I, L]
        xt = np.ascontiguousarray(xc.transpose(1, 2, 0))  # [I, L, BL] = (i, l, b)
        xt4 = xt.reshape(IB, 16, L, BL)
        xdg = np.zeros((IB, 128, 128), dtype=np.float16)
        for s in range(16):
            xdg[:, s * L : (s + 1) * L, s * BL : (s + 1) * BL] = xt4[:, s].astype(
                np.float16
            )
        xdg2 = np.ascontiguousarray(xdg.transpose(1, 0, 2))  # [128, IB, 128]
        in_maps.append({"w2": w2, "xdg": xdg2, "onesd": onesd, "biasT": biasT})

    nc = _get_nc()
    try:
        res = run_bass_kernel_spmd(nc, in_maps, core_ids=list(range(NCORES)))
    except ModuleNotFoundError:
        # BASS_TRACE was set but this environment lacks the axon NTFF hook
        # module; rerun without tracing.
        os.environ["BASS_NEVER_TRACE"] = "1"
        res = run_bass_kernel_spmd(nc, in_maps, core_ids=list(range(NCORES)))
    LAST_RESULT = res

    outs = []
    for r in res.results:
        o = r["out"]  # [BL, V, O]
        outs.append(np.ascontiguousarray(o.transpose(0, 2, 1)))  # [BL, O, V]
    return np.concatenate(outs, axis=0).astype(np.float32)


if __name__ == "__main__":
    rng = np.random.default_rng(0)
    x = rng.standard_normal((B, I, L), dtype=np.float32)
    w = rng.standard_normal((O, I, V, L), dtype=np.float32) * 0.1
    b = rng.standard_normal((O, V), dtype=np.float32) * 0.1
    out = kernel(x, w, b)
    print("out shape", out.shape, out.dtype)


# revision 19
# speedup vs baseline: 1.0409x; 1.0409x over previous
"""Trainium2 Bass kernel for nn_CapsuleLinear (k-means 'dot' routing, 3 iters).

Math (per example b):
  priors[o,i,v] = sum_l W[o,i,v,l] * x[b,i,l]
  out0 = mean_i priors
  3x: n = normalize(out); logits[o,i] = sum_v priors*n; probs = softmax_o(logits);
      out[o,v] = sum_i probs*priors
  result = squash(out) + bias

Sharding: data-parallel over batch B=64 across 8 cores (8 examples/core).

Per-core layout (P = 128 partitions = (i_p in 0..15, b in 0..7), p = i_p*8+b):
  priors SBUF fp16 [128, ib=32, v=16, o=64], full i = ib*16 + i_p.
  Phase 1: priors via PE matmuls (block-diag x as lhsT, W2[ib] as rhs);
  PSUM -> SBUF copies alternate ACT/DVE; out0 = sum_i priors accumulates
  entirely on the PE via delta_{b,b'} ones-matmuls (DVE stays empty so the
  phase is PE/DMA-bound).
  Iterations use raw logits: rawlog = priors . out (unnormalized), then one
  small scale by rn = 1/||out|| (computed as exp(-0.5 ln nsq) so the ACT
  engine never switches activation tables; softmax shift invariance is not
  affected since the scale is exact). elog/logits fp16. The out-phase
  interleaves DVE muls (8-ib chunks) with batched PE ones-matmuls so the PE
  queue stays full and ramps to its fast p-state.
"""

import os

import numpy as np

import concourse.bacc as bacc
import concourse.tile as tile
from concourse import mybir
from concourse.bass_utils import run_bass_kernel_spmd

B, I, O, V, L = 64, 512, 64, 16, 8
NCORES = 8
BL = B // NCORES  # 8 examples per core
IB = I // 16  # 32 blocks of 16 i's
IP = 16  # i_p values per partition group

f32 = mybir.dt.float32
f16 = mybir.dt.float16

LAST_RESULT = None  # stash of BassKernelResults for test harness


def _build_kernel():
    nc = bacc.Bacc(
        "TRN2",
        target_bir_lowering=False,
        debug=False,
        enable_asserts=False,
        num_devices=NCORES,
    )
    w2_d = nc.dram_tensor("w2", [IB, 128, O * V], f16, kind="ExternalInput")
    xdg_d = nc.dram_tensor("xdg", [128, IB, 128], f16, kind="ExternalInput")
    ones_d = nc.dram_tensor("onesd", [128, 128], f16, kind="ExternalInput")
    bias_d = nc.dram_tensor("biasT", [V, O], f32, kind="ExternalInput")
    out_d = nc.dram_tensor("out", [BL, V, O], f32, kind="ExternalOutput")

    with tile.TileContext(nc) as tc:
        _body(nc, tc, w2_d, xdg_d, ones_d, bias_d, out_d)
    nc.compile()
    return nc


def _body(nc, tc, w2_d, xdg_d, ones_d, bias_d, out_d):
    AL = mybir.AluOpType
    X = mybir.AxisListType.X
    AF = mybir.ActivationFunctionType

    from contextlib import ExitStack

    with ExitStack() as ctx:
        big = ctx.enter_context(tc.tile_pool(name="big", bufs=1))
        wp = ctx.enter_context(tc.tile_pool(name="wp", bufs=6))
        xp = ctx.enter_context(tc.tile_pool(name="xp", bufs=4))
        sm = ctx.enter_context(tc.tile_pool(name="sm", bufs=1))
        pr_ps = ctx.enter_context(tc.tile_pool(name="prps", bufs=3, space="PSUM"))
        out_ps = ctx.enter_context(tc.tile_pool(name="outps", bufs=1, space="PSUM"))

        # ---- persistent tiles ----
        priors = big.tile([128, IB, V, O], f16)
        prod = big.tile([128, IB, V, O], f16)
        logits = big.tile([128, IB, O], f16)
        elog = big.tile([128, IB, O], f16)
        probs = big.tile([128, IB, O], f16)
        ones_t = big.tile([128, 128], f16)
        bias_t = big.tile([BL, V, O], f32)
        out_sb = big.tile([128, V, O], f16)
        sqv = big.tile([128, V, O], f32)
        nsq = big.tile([128, O], f32)
        lnq = big.tile([128, O], f32)
        rn = big.tile([128, O], f16)
        zs = big.tile([128, IB], f32)
        rz = big.tile([128, IB], f32)

        # all 32 xdg tiles in ONE DMA (per-descriptor overhead is ~600ns, so
        # 32 small DMAs would serialize ~19us of queue time)
        xdg_all = big.tile([128, IB, 128], f16)
        nc.scalar.dma_start(out=xdg_all[:], in_=xdg_d[:])
        nc.scalar.dma_start(out=ones_t[:], in_=ones_d[:])
        nc.scalar.dma_start(
            out=bias_t[:], in_=bias_d[:].unsqueeze(0).broadcast_to([BL, V, O])
        )

        # ---- phase 1: priors + out0 ----
        # out0 = sum_i priors: ibs 0..15 via PE ones-matmuls (lagged so copy
        # deps are met), ibs 16..31 via a DVE pairwise tree (the DVE is
        # otherwise idle here); one final PE matmul pair folds the DVE
        # partial in and re-broadcasts.
        out0 = out_ps.tile([128, V, O], f32)
        out0f = out0[:].rearrange("p v o -> p (v o)")
        acc8 = big.tile([128, 8, V, O], f16)

        def out0_mms(ib_lo, ib_hi):
            for ib in range(ib_lo, ib_hi):
                pslc = priors[:, ib].rearrange("p v o -> p (v o)")
                for h in range(2):
                    sl = slice(h * 512, (h + 1) * 512)
                    nc.tensor.matmul(
                        out0f[:, sl],
                        ones_t[:],
                        pslc[:, sl],
                        start=(ib == 0),
                        stop=False,
                        skip_group_check=True,
                    )

        for ib in range(IB):
            w = wp.tile([128, O * V], f16, tag="w")
            nc.sync.dma_start(out=w[:], in_=w2_d[ib])
            pp = pr_ps.tile([128, O * V], f32, tag="pp")
            for h in range(2):
                sl = slice(h * 512, (h + 1) * 512)
                nc.tensor.matmul(
                    pp[:, sl], xdg_all[:, ib], w[:, sl], start=True, stop=True
                )
            # PSUM (o,v) -> SBUF priors[:, ib] in (v, o) order, cast to fp16;
            # alternate ACT/DVE so neither engine is the phase bottleneck.
            ppv = pp[:].rearrange("p (o v) -> p v o", o=O)
            if ib % 2 == 0:
                nc.scalar.copy(out=priors[:, ib], in_=ppv)
            else:
                nc.vector.tensor_copy(out=priors[:, ib], in_=ppv)
            if ib < 16:
                if ib % 4 == 3 and ib >= 7:
                    out0_mms(ib - 7, ib - 3)
            else:
                if ib % 2 == 1:
                    nc.vector.tensor_add(
                        acc8[:, (ib - 16) // 2], priors[:, ib - 1], priors[:, ib]
                    )
        out0_mms(12, 16)
        nc.vector.tensor_add(acc8[:, 0:4], acc8[:, 0:4], acc8[:, 4:8])
        nc.vector.tensor_add(acc8[:, 0:2], acc8[:, 0:2], acc8[:, 2:4])
        nc.vector.tensor_add(acc8[:, 0], acc8[:, 0], acc8[:, 1])
        pslc = acc8[:, 0].rearrange("p v o -> p (v o)")
        for h in range(2):
            sl = slice(h * 512, (h + 1) * 512)
            nc.tensor.matmul(
                out0f[:, sl],
                ones_t[:],
                pslc[:, sl],
                start=False,
                stop=True,
                skip_group_check=True,
            )

        # seed routing state: out_sb = out0 (scale handled by rn)
        nc.scalar.copy(out=out_sb[:], in_=out0[:])

        # ---- phase 2: routing iterations ----
        for t in range(3):
            # rawlog mul chunk 0 first so the DVE never stalls on ACT square
            nc.vector.tensor_mul(
                prod[:, 0:16],
                priors[:, 0:16],
                out_sb[:].unsqueeze(1).broadcast_to([128, 16, V, O]),
            )
            # rn = 1/||out|| via ln+exp (no activation-table switches)
            nc.scalar.square(sqv[:], out_sb[:])
            nsqv = sqv[:].transpose([0, 2, 1])  # [p, O, V] strided view
            nc.vector.tensor_reduce(out=nsq[:], in_=nsqv, axis=X, op=AL.add)
            nc.scalar.activation(out=lnq[:], in_=nsq[:], func=AF.Ln)
            nc.scalar.activation(out=rn[:], in_=lnq[:], func=AF.Exp, scale=-0.5)
            nc.vector.tensor_mul(
                prod[:, 16:32],
                priors[:, 16:32],
                out_sb[:].unsqueeze(1).broadcast_to([128, 16, V, O]),
            )
            # v-reduction tree (fp16, 2x mode)
            nc.vector.tensor_add(prod[:, :, 0:8], prod[:, :, 0:8], prod[:, :, 8:16])
            nc.vector.tensor_add(prod[:, :, 0:4], prod[:, :, 0:4], prod[:, :, 4:8])
            nc.vector.tensor_add(prod[:, :, 0:2], prod[:, :, 0:2], prod[:, :, 2:4])
            # logits = (sum_v prod) * rn ; softmax over o, split in ib-halves
            # so ACT exp overlaps the DVE tail.
            for hh in range(2):
                si = slice(hh * 16, (hh + 1) * 16)
                nc.vector.tensor_add(logits[:, si], prod[:, si, 0], prod[:, si, 1])
                nc.vector.tensor_mul(
                    logits[:, si],
                    logits[:, si],
                    rn[:].unsqueeze(1).broadcast_to([128, 16, O]),
                )
                nc.scalar.activation(out=elog[:, si], in_=logits[:, si], func=AF.Exp)
                nc.vector.tensor_reduce(
                    out=zs[:, si], in_=elog[:, si], axis=X, op=AL.add
                )
                nc.vector.reciprocal(rz[:, si], zs[:, si])
                nc.vector.tensor_mul(
                    probs[:, si],
                    elog[:, si],
                    rz[:, si].unsqueeze(2).broadcast_to([128, 16, O]),
                )

            # out_new[p, v, o] = sum_i probs * priors: DVE muls in 8-ib
            # chunks, PE ones-matmuls batched per chunk (PE queue stays full
            # -> fast p-state; it rides ~2us behind the DVE).
            out_new = pr_ps.tile([128, V, O], f32, tag="pp")
            onf = out_new[:].rearrange("p v o -> p (v o)")
            # chunk sizes taper so the PE's last matmul batch lands right
            # after the DVE's last mul (small tail)
            bounds = [0, 12, 22, 28, 32]
            for c in range(4):
                lo, hi = bounds[c], bounds[c + 1]
                s = slice(lo, hi)
                nc.vector.tensor_mul(
                    prod[:, s],
                    priors[:, s],
                    probs[:, s].unsqueeze(2).broadcast_to([128, hi - lo, V, O]),
                )
                for ib in range(lo, hi):
                    pslc = prod[:, ib].rearrange("p v o -> p (v o)")
                    for h in range(2):
                        sl = slice(h * 512, (h + 1) * 512)
                        nc.tensor.matmul(
                            onf[:, sl],
                            ones_t[:],
                            pslc[:, sl],
                            start=(ib == 0),
                            stop=(ib == IB - 1),
                            skip_group_check=True,
                        )
            if t < 2:
                nc.scalar.copy(out=out_sb[:], in_=out_new[:])
            else:
                out_prev = out_new

        # ---- squash + bias on partitions 0..7 (b rows) ----
        out3 = sm.tile([BL, V, O], f32, tag="out3")
        nc.scalar.copy(out=out3[:], in_=out_prev[0:BL])
        nc.scalar.square(sqv[0:BL], out3[:])
        nsqv3 = sqv[0:BL].transpose([0, 2, 1])
        nc.vector.tensor_reduce(out=nsq[0:BL], in_=nsqv3, axis=X, op=AL.add)
        nc.scalar.activation(out=lnq[0:BL], in_=nsq[0:BL], func=AF.Ln)
        norm3 = sm.tile([BL, O], f32, tag="norm3")
        nc.scalar.activation(out=norm3[:], in_=lnq[0:BL], func=AF.Exp, scale=0.5)
        den = sm.tile([BL, O], f32, tag="den")
        nc.vector.tensor_scalar_add(den[:], nsq[0:BL], 1.0)
        rden = sm.tile([BL, O], f32, tag="rden")
        nc.vector.reciprocal(rden[:], den[:])
        scl = sm.tile([BL, O], f32, tag="scl")
        nc.vector.tensor_mul(scl[:], norm3[:], rden[:])

        outf = sm.tile([BL, V, O], f32, tag="outf")
        nc.vector.tensor_mul(
            outf[:], out3[:], scl[:].unsqueeze(1).broadcast_to([BL, V, O])
        )
        nc.vector.tensor_add(outf[:], outf[:], bias_t[:])
        nc.sync.dma_start(out=out_d[:], in_=outf[:])


_NC_CACHE = []


def _get_nc():
    if not _NC_CACHE:
        _NC_CACHE.append(_build_kernel())
    return _NC_CACHE[0]


def kernel(x, weight, bias):
    global LAST_RESULT
    x = np.asarray(x, dtype=np.float32)
    weight = np.asarray(weight, dtype=np.float32)
    bias = np.asarray(bias, dtype=np.float32)

    # W2[ib, (i_sub, l), (o, v)] = W[o, ib*16+i_sub, v, l]  (fp16: same byte
    # cost as bf16 but 4x finer mantissa; values are well within fp16 range)
    w2 = (
        np.ascontiguousarray(weight.transpose(1, 3, 0, 2))
        .reshape(IB, 128, O * V)
        .astype(np.float16)
    )
    biasT = np.ascontiguousarray(bias.T)  # [V, O]

    idx = np.arange(128)
    onesd = (idx[:, None] % BL == idx[None, :] % BL).astype(np.float16)

    in_maps = []
    for c in range(NCORES):
        xc = x[c * BL : (c + 1) * BL]  # [BL, I, L]
        xt = np.ascontiguousarray(xc.transpose(1, 2, 0))  # [I, L, BL] = (i, l, b)
        xt4 = xt.reshape(IB, 16, L, BL)
        xdg = np.zeros((IB, 128, 128), dtype=np.float16)
        for s in range(16):
            xdg[:, s * L : (s + 1) * L, s * BL : (s + 1) * BL] = xt4[:, s].astype(
                np.float16
            )
        xdg2 = np.ascontiguousarray(xdg.transpose(1, 0, 2))  # [128, IB, 128]
        in_maps.append({"w2": w2, "xdg": xdg2, "onesd": onesd, "biasT": biasT})

    nc = _get_nc()
    try:
        res = run_bass_kernel_spmd(nc, in_maps, core_ids=list(range(NCORES)))
    except ModuleNotFoundError:
        # BASS_TRACE was set but this environment lacks the axon NTFF hook
        # module; rerun without tracing.
        os.environ["BASS_NEVER_TRACE"] = "1"
        res = run_bass_kernel_spmd(nc, in_maps, core_ids=list(range(NCORES)))
    LAST_RESULT = res

    outs = []
    for r in res.results:
        o = r["out"]  # [BL, V, O]
        outs.append(np.ascontiguousarray(o.transpose(0, 2, 1)))  # [BL, O, V]
    return np.concatenate(outs, axis=0).astype(np.float32)


if __name__ == "__main__":
    rng = np.random.default_rng(0)
    x = rng.standard_normal((B, I, L), dtype=np.float32)
    w = rng.standard_normal((O, I, V, L), dtype=np.float32) * 0.1
    b = rng.standard_normal((O, V), dtype=np.float32) * 0.1
    out = kernel(x, w, b)
    print("out shape", out.shape, out.dtype)


# revision 20
# speedup vs baseline: 1.2074x; 1.1599x over previous
"""Trainium2 Bass kernel for nn_CapsuleLinear (k-means 'dot' routing, 3 iters).

Math (per example b):
  priors[o,i,v] = sum_l W[o,i,v,l] * x[b,i,l]
  out0 = mean_i priors
  3x: n = normalize(out); logits[o,i] = sum_v priors*n; probs = softmax_o(logits);
      out[o,v] = sum_i probs*priors
  result = squash(out) + bias

Sharding: data-parallel over batch B=64 across 8 cores (8 examples/core).

Per-core layout (P = 128 partitions = (i_p in 0..15, b in 0..7), p = i_p*8+b):
  priors SBUF fp16 [128, ib=32, v=16, o=64], full i = ib*16 + i_p.
  Phase 1: priors via PE matmuls (block-diag x as lhsT, W2[ib] as rhs);
  PSUM -> SBUF copies alternate ACT/DVE; out0 = sum_i priors accumulates
  entirely on the PE via delta_{b,b'} ones-matmuls (DVE stays empty so the
  phase is PE/DMA-bound).
  Iterations use raw logits: rawlog = priors . out (unnormalized), then one
  small scale by rn = 1/||out|| (computed as exp(-0.5 ln nsq) so the ACT
  engine never switches activation tables; softmax shift invariance is not
  affected since the scale is exact). elog/logits fp16. The out-phase
  interleaves DVE muls (8-ib chunks) with batched PE ones-matmuls so the PE
  queue stays full and ramps to its fast p-state.
"""

import os

import numpy as np

import concourse.bacc as bacc
import concourse.tile as tile
from concourse import mybir
from concourse.bass_utils import run_bass_kernel_spmd

B, I, O, V, L = 64, 512, 64, 16, 8
NCORES = 8
BL = B // NCORES  # 8 examples per core
IB = I // 16  # 32 blocks of 16 i's
IP = 16  # i_p values per partition group

f32 = mybir.dt.float32
f16 = mybir.dt.float16

LAST_RESULT = None  # stash of BassKernelResults for test harness


def _build_kernel():
    nc = bacc.Bacc(
        "TRN2",
        target_bir_lowering=False,
        debug=False,
        enable_asserts=False,
        num_devices=NCORES,
    )
    w2_d = nc.dram_tensor("w2", [IB, 128, O * V], f16, kind="ExternalInput")
    xdg_d = nc.dram_tensor("xdg", [128, IB, 128], f16, kind="ExternalInput")
    ones_d = nc.dram_tensor("onesd", [128, 128], f16, kind="ExternalInput")
    bias_d = nc.dram_tensor("biasT", [V, O], f32, kind="ExternalInput")
    out_d = nc.dram_tensor("out", [BL, V, O], f32, kind="ExternalOutput")

    with tile.TileContext(nc) as tc:
        _body(nc, tc, w2_d, xdg_d, ones_d, bias_d, out_d)
    nc.compile()
    return nc


def _body(nc, tc, w2_d, xdg_d, ones_d, bias_d, out_d):
    AL = mybir.AluOpType
    X = mybir.AxisListType.X
    AF = mybir.ActivationFunctionType

    from contextlib import ExitStack

    with ExitStack() as ctx:
        big = ctx.enter_context(tc.tile_pool(name="big", bufs=1))
        wp = ctx.enter_context(tc.tile_pool(name="wp", bufs=6))
        xp = ctx.enter_context(tc.tile_pool(name="xp", bufs=4))
        sm = ctx.enter_context(tc.tile_pool(name="sm", bufs=1))
        pr_ps = ctx.enter_context(tc.tile_pool(name="prps", bufs=3, space="PSUM"))
        out_ps = ctx.enter_context(tc.tile_pool(name="outps", bufs=1, space="PSUM"))

        # ---- persistent tiles ----
        priors = big.tile([128, IB, V, O], f16)
        prod = big.tile([128, IB, V, O], f16)
        logits = big.tile([128, IB, O], f16)
        elog = big.tile([128, IB, O], f16)
        probs = big.tile([128, IB, O], f16)
        ones_t = big.tile([128, 128], f16)
        bias_t = big.tile([BL, V, O], f32)
        out_sb = big.tile([128, V, O], f16)
        sqv = big.tile([128, V, O], f32)
        nsq = big.tile([128, O], f32)
        lnq = big.tile([128, O], f32)
        rn = big.tile([128, O], f16)
        zs = big.tile([128, IB], f32)
        rz = big.tile([128, IB], f32)

        # all 32 xdg tiles in ONE DMA (per-descriptor overhead is ~600ns, so
        # 32 small DMAs would serialize ~19us of queue time)
        xdg_all = big.tile([128, IB, 128], f16)
        nc.scalar.dma_start(out=xdg_all[:], in_=xdg_d[:])
        nc.scalar.dma_start(out=ones_t[:], in_=ones_d[:])
        nc.scalar.dma_start(
            out=bias_t[:], in_=bias_d[:].unsqueeze(0).broadcast_to([BL, V, O])
        )

        # ---- phase 1: priors + out0 ----
        # out0 = sum_i priors: ibs 0..15 via PE ones-matmuls (lagged so copy
        # deps are met), ibs 16..31 via a DVE pairwise tree (the DVE is
        # otherwise idle here); one final PE matmul pair folds the DVE
        # partial in and re-broadcasts.
        out0 = out_ps.tile([128, V, O], f32)
        out0f = out0[:].rearrange("p v o -> p (v o)")
        acc8 = big.tile([128, 8, V, O], f16)

        def out0_mms(ib_lo, ib_hi):
            for ib in range(ib_lo, ib_hi):
                pslc = priors[:, ib].rearrange("p v o -> p (v o)")
                for h in range(2):
                    sl = slice(h * 512, (h + 1) * 512)
                    nc.tensor.matmul(
                        out0f[:, sl],
                        ones_t[:],
                        pslc[:, sl],
                        start=(ib == 0),
                        stop=False,
                        skip_group_check=True,
                    )

        for ib in range(IB):
            w = wp.tile([128, O * V], f16, tag="w")
            nc.sync.dma_start(out=w[:], in_=w2_d[ib])
            pp = pr_ps.tile([128, O * V], f32, tag="pp")
            for h in range(2):
                sl = slice(h * 512, (h + 1) * 512)
                nc.tensor.matmul(
                    pp[:, sl], xdg_all[:, ib], w[:, sl], start=True, stop=True
                )
            # PSUM (o,v) -> SBUF priors[:, ib] in (v, o) order, cast to fp16;
            # alternate ACT/DVE so neither engine is the phase bottleneck.
            ppv = pp[:].rearrange("p (o v) -> p v o", o=O)
            if ib % 2 == 0:
                nc.scalar.copy(out=priors[:, ib], in_=ppv)
            else:
                nc.vector.tensor_copy(out=priors[:, ib], in_=ppv)
            if ib < 16:
                if ib % 4 == 3 and ib >= 7:
                    out0_mms(ib - 7, ib - 3)
            elif ib >= 19 and ib % 2 == 1:
                # pair-add lagged 2 ibs so the DVE queue head never blocks
                # on a fresh ACT copy (head-of-line stall would back up the
                # psum pool and stall the PE).
                j = ib - 3
                nc.vector.tensor_add(
                    acc8[:, (j - 16) // 2], priors[:, j], priors[:, j + 1]
                )
        out0_mms(12, 16)
        nc.vector.tensor_add(acc8[:, 7], priors[:, 30], priors[:, 31])
        nc.vector.tensor_add(acc8[:, 0:4], acc8[:, 0:4], acc8[:, 4:8])
        nc.vector.tensor_add(acc8[:, 0:2], acc8[:, 0:2], acc8[:, 2:4])
        nc.vector.tensor_add(acc8[:, 0], acc8[:, 0], acc8[:, 1])
        pslc = acc8[:, 0].rearrange("p v o -> p (v o)")
        for h in range(2):
            sl = slice(h * 512, (h + 1) * 512)
            nc.tensor.matmul(
                out0f[:, sl],
                ones_t[:],
                pslc[:, sl],
                start=False,
                stop=True,
                skip_group_check=True,
            )

        # seed routing state: out_sb = out0 (scale handled by rn)
        nc.scalar.copy(out=out_sb[:], in_=out0[:])

        # ---- phase 2: routing iterations ----
        for t in range(3):
            # rawlog mul chunk 0 first so the DVE never stalls on ACT square
            nc.vector.tensor_mul(
                prod[:, 0:16],
                priors[:, 0:16],
                out_sb[:].unsqueeze(1).broadcast_to([128, 16, V, O]),
            )
            # rn = 1/||out|| via ln+exp (no activation-table switches)
            nc.scalar.square(sqv[:], out_sb[:])
            nsqv = sqv[:].transpose([0, 2, 1])  # [p, O, V] strided view
            nc.vector.tensor_reduce(out=nsq[:], in_=nsqv, axis=X, op=AL.add)
            nc.scalar.activation(out=lnq[:], in_=nsq[:], func=AF.Ln)
            nc.scalar.activation(out=rn[:], in_=lnq[:], func=AF.Exp, scale=-0.5)
            nc.vector.tensor_mul(
                prod[:, 16:32],
                priors[:, 16:32],
                out_sb[:].unsqueeze(1).broadcast_to([128, 16, V, O]),
            )
            # v-reduction tree (fp16, 2x mode)
            nc.vector.tensor_add(prod[:, :, 0:8], prod[:, :, 0:8], prod[:, :, 8:16])
            nc.vector.tensor_add(prod[:, :, 0:4], prod[:, :, 0:4], prod[:, :, 4:8])
            nc.vector.tensor_add(prod[:, :, 0:2], prod[:, :, 0:2], prod[:, :, 2:4])
            # logits = (sum_v prod) * rn ; softmax over o, split in ib-halves
            # so ACT exp overlaps the DVE tail.
            for hh in range(2):
                si = slice(hh * 16, (hh + 1) * 16)
                nc.vector.tensor_add(logits[:, si], prod[:, si, 0], prod[:, si, 1])
                nc.vector.tensor_mul(
                    logits[:, si],
                    logits[:, si],
                    rn[:].unsqueeze(1).broadcast_to([128, 16, O]),
                )
                nc.scalar.activation(out=elog[:, si], in_=logits[:, si], func=AF.Exp)
                nc.vector.tensor_reduce(
                    out=zs[:, si], in_=elog[:, si], axis=X, op=AL.add
                )
                nc.vector.reciprocal(rz[:, si], zs[:, si])
                nc.vector.tensor_mul(
                    probs[:, si],
                    elog[:, si],
                    rz[:, si].unsqueeze(2).broadcast_to([128, 16, O]),
                )

            # out_new[p, v, o] = sum_i probs * priors: DVE muls in 8-ib
            # chunks, PE ones-matmuls batched per chunk (PE queue stays full
            # -> fast p-state; it rides ~2us behind the DVE).
            out_new = pr_ps.tile([128, V, O], f32, tag="pp")
            onf = out_new[:].rearrange("p v o -> p (v o)")
            # chunk sizes taper so the PE's last matmul batch lands right
            # after the DVE's last mul (small tail)
            bounds = [0, 12, 22, 28, 32]
            for c in range(4):
                lo, hi = bounds[c], bounds[c + 1]
                s = slice(lo, hi)
                nc.vector.tensor_mul(
                    prod[:, s],
                    priors[:, s],
                    probs[:, s].unsqueeze(2).broadcast_to([128, hi - lo, V, O]),
                )
                for ib in range(lo, hi):
                    pslc = prod[:, ib].rearrange("p v o -> p (v o)")
                    for h in range(2):
                        sl = slice(h * 512, (h + 1) * 512)
                        nc.tensor.matmul(
                            onf[:, sl],
                            ones_t[:],
                            pslc[:, sl],
                            start=(ib == 0),
                            stop=(ib == IB - 1),
                            skip_group_check=True,
                        )
            if t < 2:
                nc.scalar.copy(out=out_sb[:], in_=out_new[:])
            else:
                out_prev = out_new

        # ---- squash + bias on partitions 0..7 (b rows) ----
        out3 = sm.tile([BL, V, O], f32, tag="out3")
        nc.scalar.copy(out=out3[:], in_=out_prev[0:BL])
        nc.scalar.square(sqv[0:BL], out3[:])
        nsqv3 = sqv[0:BL].transpose([0, 2, 1])
        nc.vector.tensor_reduce(out=nsq[0:BL], in_=nsqv3, axis=X, op=AL.add)
        nc.scalar.activation(out=lnq[0:BL], in_=nsq[0:BL], func=AF.Ln)
        norm3 = sm.tile([BL, O], f32, tag="norm3")
        nc.scalar.activation(out=norm3[:], in_=lnq[0:BL], func=AF.Exp, scale=0.5)
        den = sm.tile([BL, O], f32, tag="den")
        nc.vector.tensor_scalar_add(den[:], nsq[0:BL], 1.0)
        rden = sm.tile([BL, O], f32, tag="rden")
        nc.vector.reciprocal(rden[:], den[:])
        scl = sm.tile([BL, O], f32, tag="scl")
        nc.vector.tensor_mul(scl[:], norm3[:], rden[:])

        outf = sm.tile([BL, V, O], f32, tag="outf")
        nc.vector.tensor_mul(
            outf[:], out3[:], scl[:].unsqueeze(1).broadcast_to([BL, V, O])
        )
        nc.vector.tensor_add(outf[:], outf[:], bias_t[:])
        nc.sync.dma_start(out=out_d[:], in_=outf[:])


_NC_CACHE = []


def _get_nc():
    if not _NC_CACHE:
        _NC_CACHE.append(_build_kernel())
    return _NC_CACHE[0]


def kernel(x, weight, bias):
    global LAST_RESULT
    x = np.asarray(x, dtype=np.float32)
    weight = np.asarray(weight, dtype=np.float32)
    bias = np.asarray(bias, dtype=np.float32)

    # W2[ib, (i_sub, l), (o, v)] = W[o, ib*16+i_sub, v, l]  (fp16: same byte
    # cost as bf16 but 4x finer mantissa; values are well within fp16 range)
    w2 = (
        np.ascontiguousarray(weight.transpose(1, 3, 0, 2))
        .reshape(IB, 128, O * V)
        .astype(np.float16)
    )
    biasT = np.ascontiguousarray(bias.T)  # [V, O]

    idx = np.arange(128)
    onesd = (idx[:, None] % BL == idx[None, :] % BL).astype(np.float16)

    in_maps = []
    for c in range(NCORES):
        xc = x[c * BL : (c + 1) * BL]  # [BL, I, L]
        xt = np.ascontiguousarray(xc.transpose(1, 2, 0))  # [I, L, BL] = (i, l, b)
        xt4 = xt.reshape(IB, 16, L, BL)
        xdg = np.zeros((IB, 128, 128), dtype=np.float16)
        for s in range(16):
            xdg[:, s * L : (s + 1) * L, s * BL : (s + 1) * BL] = xt4[:, s].astype(
                np.float16
            )
        xdg2 = np.ascontiguousarray(xdg.transpose(1, 0, 2))  # [128, IB, 128]
        in_maps.append({"w2": w2, "xdg": xdg2, "onesd": onesd, "biasT": biasT})

    nc = _get_nc()
    try:
        res = run_bass_kernel_spmd(nc, in_maps, core_ids=list(range(NCORES)))
    except ModuleNotFoundError:
        # BASS_TRACE was set but this environment lacks the axon NTFF hook
        # module; rerun without tracing.
        os.environ["BASS_NEVER_TRACE"] = "1"
        res = run_bass_kernel_spmd(nc, in_maps, core_ids=list(range(NCORES)))
    LAST_RESULT = res

    outs = []
    for r in res.results:
        o = r["out"]  # [BL, V, O]
        outs.append(np.ascontiguousarray(o.transpose(0, 2, 1)))  # [BL, O, V]
    return np.concatenate(outs, axis=0).astype(np.float32)


if __name__ == "__main__":
    rng = np.random.default_rng(0)
    x = rng.standard_normal((B, I, L), dtype=np.float32)
    w = rng.standard_normal((O, I, V, L), dtype=np.float32) * 0.1
    b = rng.standard_normal((O, V), dtype=np.float32) * 0.1
    out = kernel(x, w, b)
    print("out shape", out.shape, out.dtype)
